# revision 7
# baseline (speedup 1.0000x reference)
"""Causal self-attention (GQA + RoPE) on 8 trn2 NeuronCores via Bass/Tile.

Sharding v2: core c = (g, bh) with g = c//2 the kv-group and bh = c%2 the
batch half. Each core projects Q for its 4 q-heads and K/V for its kv head,
over its 2 batches only — so K/V projections are computed exactly once
across the mesh (the old layout duplicated them 2x) and the x DMA halves.
o_proj stays token-parallel in a second kernel (host reslices y between the
two launches; all FLOPs on device).

Numerics: bf16 operands into the PE everywhere (same 1 cycle/row as f32r on
TRN2, but half the SBUF/DMA bytes and 2x DVE throughput), fp32 PSUM
accumulation. Softmax without max-subtraction. Softmax denominators: exp
tiles are tree-summed in groups of 4 on the DVE (bf16, 2x mode) and only one
ones-matmul per group hits the PE (vs one per tile before) — the PE was the
bottleneck engine, the DVE has slack.

Shapes hardcoded for B=4, T=2048, D=2048, 16 heads x 128, 4 kv heads x 128.
"""
import numpy as np
import ml_dtypes

import concourse.bacc as bacc
import concourse.mybir as mybir
from concourse.tile import TileContext
from concourse.bass_utils import run_bass_kernel_spmd

N_CORES = 8
B, T, D = 4, 2048, 2048
N_HEAD, N_KV, HD = 16, 4, 128
NTOK = B * T                      # 8192
CHUNK = 512
TOK_PER_CORE = NTOK // N_CORES    # 1024
TB = 2 * T                        # tokens per core in kernel A (2 batches)
SCALE = float(1.0 / np.sqrt(128.0))
ROPE_THETA = 10000.0

F32 = mybir.dt.float32
F32R = mybir.dt.float32r
BF16 = mybir.dt.bfloat16
BF = ml_dtypes.bfloat16


def build_kernel_a():
    nc = bacc.Bacc("TRN2", target_bir_lowering=False, debug=False,
                   num_devices=N_CORES, name="attn_a")
    xT = nc.dram_tensor("xT", [128, 16, TB], BF16, kind="ExternalInput")
    wq = nc.dram_tensor("wq", [128, 16, 512], BF16, kind="ExternalInput")
    wk = nc.dram_tensor("wk", [128, 16, 128], BF16, kind="ExternalInput")
    wv = nc.dram_tensor("wv", [128, 16, 128], BF16, kind="ExternalInput")
    cosT = nc.dram_tensor("cosT", [128, T], F32, kind="ExternalInput")
    sinM = nc.dram_tensor("sinM", [128, T], F32, kind="ExternalInput")
    maskW = nc.dram_tensor("maskW", [128, 896], BF16, kind="ExternalInput")
    ident_in = nc.dram_tensor("ident_in", [128, 128], F32, kind="ExternalInput")
    ones_in = nc.dram_tensor("ones_in", [128, 1], BF16, kind="ExternalInput")
    onesr_in = nc.dram_tensor("onesr_in", [1, 128], F32R, kind="ExternalInput")
    y = nc.dram_tensor("y", [512, TB], BF16, kind="ExternalOutput")

    with TileContext(nc) as tc:
        with tc.tile_pool(name="wpool", bufs=1) as wpool, \
             tc.tile_pool(name="xpool", bufs=8) as xpool, \
             tc.tile_pool(name="tpool", bufs=3) as tpool, \
             tc.tile_pool(name="qkv", bufs=2) as qkv, \
             tc.tile_pool(name="ep", bufs=8) as ep, \
             tc.tile_pool(name="gp", bufs=6) as gp, \
             tc.tile_pool(name="yu", bufs=4) as yu, \
             tc.tile_pool(name="su", bufs=4) as su, \
             tc.tile_pool(name="yp", bufs=3) as yp, \
             tc.tile_pool(name="psum", bufs=1, space="PSUM") as pp:
            # DMA issue order = service order: first proj (K of batch 0)
            # needs wk + the first x chunk, so those go first.
            wk_sb = wpool.tile([128, 16, 128], BF16)
            nc.sync.dma_start(out=wk_sb[:], in_=wk[:])
            xq0 = []
            for qtr in range(4):
                t = xpool.tile([128, 4, CHUNK], BF16, name="xq")
                nc.sync.dma_start(out=t[:], in_=xT[:, 4 * qtr:4 * qtr + 4, 0:CHUNK])
                xq0.append(t)
            wq_sb = wpool.tile([128, 16, 512], BF16)
            nc.sync.dma_start(out=wq_sb[:], in_=wq[:])
            wv_sb = wpool.tile([128, 16, 128], BF16)
            nc.sync.dma_start(out=wv_sb[:], in_=wv[:])
            cos_sb = wpool.tile([128, T], F32)
            nc.sync.dma_start(out=cos_sb[:], in_=cosT[:])
            sin_sb = wpool.tile([128, T], F32)
            nc.sync.dma_start(out=sin_sb[:], in_=sinM[:])
            id_sb = wpool.tile([128, 128], F32)
            nc.sync.dma_start(out=id_sb[:], in_=ident_in[:])
            mask_sb = wpool.tile([128, 896], BF16)
            nc.sync.dma_start(out=mask_sb[:], in_=maskW[:])
            ones_sb = wpool.tile([128, 1], BF16)
            nc.sync.dma_start(out=ones_sb[:], in_=ones_in[:])
            onesr_sb = wpool.tile([1, 128], F32R)
            nc.sync.dma_start(out=onesr_sb[:], in_=onesr_in[:])

            pending = []

            def flush_one():
                y_u, rrow, h, b, qc = pending.pop(0)
                col0 = b * T + qc * CHUNK
                b_ps = pp.tile([128, CHUNK], F32, name="b_ps", bufs=1)
                nc.tensor.matmul(b_ps[:], onesr_sb[:], rrow[:], start=True, stop=True)
                y_sb = yp.tile([128, CHUNK], BF16, name="y_sb")
                nc.vector.tensor_mul(y_sb[:], y_u[:], b_ps[:])
                nc.sync.dma_start(out=y[h * 128:(h + 1) * 128, col0:col0 + CHUNK],
                                  in_=y_sb[:])

            for b in range(2):
                # ---- projections + rope for local batch b ----
                qb = [qkv.tile([128, T], BF16, name=f"qb{h}") for h in range(4)]
                kb = qkv.tile([128, T], BF16, name="kb")
                vtb = qkv.tile([128, 16, 128], BF16, name="vtb")
                for cc in range(4):
                    c0 = b * T + cc * CHUNK
                    tcol = cc * CHUNK
                    if b == 0 and cc == 0:
                        xq = xq0
                    else:
                        xq = []
                        for qtr in range(4):
                            t = xpool.tile([128, 4, CHUNK], BF16, name="xq")
                            nc.sync.dma_start(
                                out=t[:], in_=xT[:, 4 * qtr:4 * qtr + 4, c0:c0 + CHUNK])
                            xq.append(t)

                    def proj(w_sb, off):
                        ps2 = pp.tile([128, 2 * CHUNK], F32, name="s2", bufs=2)
                        ps = ps2[:, 0:CHUNK]
                        for kt in range(16):
                            nc.tensor.matmul(ps, w_sb[:, kt, off:off + 128],
                                             xq[kt // 4][:, kt % 4, :],
                                             start=(kt == 0), stop=(kt == 15))
                        return ps

                    def rope(ps, dst):
                        # DVE reads the proj psum directly (fp32, 1x) exactly
                        # like the proven baseline; only dst is bf16.
                        t1 = tpool.tile([128, CHUNK], F32, name="t1")
                        t2 = tpool.tile([128, CHUNK], F32, name="t2")
                        nc.vector.tensor_mul(t1[:], ps[:], cos_sb[:, tcol:tcol + CHUNK])
                        nc.vector.tensor_mul(t2[0:64, :], ps[64:128, :],
                                             sin_sb[0:64, tcol:tcol + CHUNK])
                        nc.vector.tensor_mul(t2[64:128, :], ps[0:64, :],
                                             sin_sb[64:128, tcol:tcol + CHUNK])
                        nc.vector.tensor_add(dst, t1[:], t2[:])

                    rope(proj(wk_sb, 0), kb[:, tcol:tcol + CHUNK])

                    ps_v = proj(wv_sb, 0)
                    vtmp = tpool.tile([128, CHUNK], F32, name="vtmp")
                    nc.scalar.copy(vtmp[:], ps_v)
                    pt = pp.tile([128, CHUNK], F32, name="b_ps", bufs=1)
                    for j in range(4):
                        nc.tensor.transpose(pt[:, j * 128:(j + 1) * 128],
                                            vtmp[:, j * 128:(j + 1) * 128], id_sb[:])
                    for j in range(4):
                        nc.scalar.copy(vtb[:, 4 * cc + j, :], pt[:, j * 128:(j + 1) * 128])

                    for h in range(4):
                        rope(proj(wq_sb, 128 * h), qb[h][:, tcol:tcol + CHUNK])

                # ---- attention for local batch b ----
                for h in range(4):
                    for qc in range(4):
                        while len(pending) > 2:
                            flush_one()
                        y_ps = pp.tile([128, CHUNK], F32, name="y_ps", bufs=2)
                        sum_ps = pp.tile([1, CHUNK], F32, name="sum_ps", bufs=1)
                        for grp in range(qc + 1):
                            e2s = []
                            for p in range(2):
                                s2 = pp.tile([128, 2 * CHUNK], F32, name="s2", bufs=2)
                                e2 = ep.tile([128, 2 * CHUNK], BF16, name="e_sb")
                                for jj in range(2):
                                    kt = 4 * grp + 2 * p + jj
                                    nc.tensor.matmul(
                                        s2[:, jj * CHUNK:(jj + 1) * CHUNK],
                                        kb[:, kt * 128:(kt + 1) * 128],
                                        qb[h][:, qc * CHUNK:(qc + 1) * CHUNK],
                                        start=True, stop=True)
                                nc.scalar.activation(e2[:], s2[:],
                                                     mybir.ActivationFunctionType.Exp,
                                                     bias=0.0, scale=SCALE)
                                for jj in range(2):
                                    kt = 4 * grp + 2 * p + jj
                                    j = 2 * p + jj
                                    eh = e2[:, jj * CHUNK:(jj + 1) * CHUNK]
                                    if grp == qc:
                                        off = 384 - 128 * j
                                        nc.vector.tensor_mul(eh, eh,
                                                             mask_sb[:, off:off + CHUNK])
                                    nc.tensor.matmul(y_ps[:], vtb[:, kt, :], eh,
                                                     start=(kt == 0),
                                                     stop=(kt == 4 * qc + 3))
                                e2s.append(e2)
                            # group denominator: halves-adds on the idle Pool
                            # engine (off the exp->PV critical path), final
                            # combine on DVE, one ones-matmul per group.
                            g01 = gp.tile([128, CHUNK], BF16, name="g")
                            nc.gpsimd.tensor_add(g01[:], e2s[0][:, 0:CHUNK],
                                                 e2s[0][:, CHUNK:2 * CHUNK])
                            g23 = gp.tile([128, CHUNK], BF16, name="g")
                            nc.gpsimd.tensor_add(g23[:], e2s[1][:, 0:CHUNK],
                                                 e2s[1][:, CHUNK:2 * CHUNK])
                            gs = gp.tile([128, CHUNK], BF16, name="g")
                            nc.vector.tensor_add(gs[:], g01[:], g23[:])
                            nc.tensor.matmul(sum_ps[:], ones_sb[:], gs[:],
                                             start=(grp == 0), stop=(grp == qc))
                        y_u = yu.tile([128, CHUNK], BF16, name="y_u")
                        nc.scalar.copy(y_u[:], y_ps[:])
                        lrow = su.tile([1, CHUNK], F32, name="lrow", bufs=2)
                        nc.vector.reciprocal_approx_fast(out=lrow[:], in_=sum_ps[:])
                        rrow = su.tile([1, CHUNK], F32R, name="rrow")
                        nc.scalar.copy(rrow[:], lrow[:])
                        pending.append((y_u, rrow, h, b, qc))
            while pending:
                flush_one()
    nc.compile()
    return nc


def build_kernel_b():
    nc = bacc.Bacc("TRN2", target_bir_lowering=False, debug=False,
                   num_devices=N_CORES, name="attn_b")
    ya = nc.dram_tensor("ya", [128, 16, TOK_PER_CORE], BF16, kind="ExternalInput")
    wo = nc.dram_tensor("wo", [128, 16, D], BF16, kind="ExternalInput")
    outp = nc.dram_tensor("outp", [TOK_PER_CORE, D], F32, kind="ExternalOutput")
    NTT = TOK_PER_CORE // 128          # 8
    with TileContext(nc) as tc:
        with tc.tile_pool(name="yap", bufs=1) as yap, \
             tc.tile_pool(name="wop", bufs=18) as wop, \
             tc.tile_pool(name="obp", bufs=3) as obp, \
             tc.tile_pool(name="pb", bufs=4, space="PSUM") as pb:
            # fine-grained DMA: first matmul only needs ya tile 0 + one
            # [128,512] weight slice (~0.6 MB) instead of 1.5 MB -> PE
            # starts ~10us earlier.
            ya_t = []
            t = yap.tile([128, 16, 128], BF16, name="yat0")
            nc.sync.dma_start(out=t[:], in_=ya[:, :, 0:128])
            ya_t.append(t)
            w0 = []
            for kt in range(16):
                w = wop.tile([128, 512], BF16, name="wt")
                nc.sync.dma_start(out=w[:], in_=wo[:, kt, 0:512])
                w0.append(w)
                if kt < NTT - 1:
                    t = yap.tile([128, 16, 128], BF16, name=f"yat{kt + 1}")
                    nc.sync.dma_start(out=t[:],
                                      in_=ya[:, :, (kt + 1) * 128:(kt + 2) * 128])
                    ya_t.append(t)
            for oc in range(4):
                if oc == 0:
                    wt = w0
                else:
                    wt = []
                    for kt in range(16):
                        w = wop.tile([128, 512], BF16, name="wt")
                        nc.sync.dma_start(out=w[:],
                                          in_=wo[:, kt, oc * 512:(oc + 1) * 512])
                        wt.append(w)
                for tt in range(NTT):
                    ps = pb.tile([128, 512], F32, name="ps")
                    for kt in range(16):
                        nc.tensor.matmul(ps[:], ya_t[tt][:, kt, :], wt[kt][:],
                                         start=(kt == 0), stop=(kt == 15))
                    ob = obp.tile([128, 512], F32, name="ob")
                    nc.scalar.copy(ob[:], ps[:])
                    nc.sync.dma_start(
                        out=outp[tt * 128:(tt + 1) * 128, oc * 512:(oc + 1) * 512],
                        in_=ob[:])
    nc.compile()
    return nc


_cache = {}


def _get_kernels():
    if "a" not in _cache:
        _cache["a"] = build_kernel_a()
        _cache["b"] = build_kernel_b()
    return _cache["a"], _cache["b"]


def _to_pkto(w):
    # (D, O) -> (128, D//128, O): partition-major layout matching SBUF tiles
    Dd, O = w.shape
    return np.ascontiguousarray(w.reshape(Dd // 128, 128, O).transpose(1, 0, 2))


def _prep_inputs(x, position_ids, Wq, Wk, Wv, Wo):
    x = np.ascontiguousarray(np.asarray(x), dtype=np.float32)
    pos = np.asarray(position_ids).astype(np.float32)
    Wq = np.asarray(Wq, dtype=np.float32)
    Wk = np.asarray(Wk, dtype=np.float32)
    Wv = np.asarray(Wv, dtype=np.float32)
    Wo = np.asarray(Wo, dtype=np.float32)

    xr = x.reshape(NTOK, D).T.reshape(16, 128, NTOK).transpose(1, 0, 2)  # [128,16,NTOK]

    inv = (1.0 / (ROPE_THETA ** (np.arange(0, HD, 2, dtype=np.float32) / HD))).astype(np.float32)
    freqs = np.outer(pos, inv).astype(np.float32)          # (T, 64)
    emb = np.concatenate([freqs, freqs], axis=1)           # (T, 128)
    cosT = np.ascontiguousarray(np.cos(emb).T)             # (128, T)
    sinT = np.sin(emb).T
    sign = np.where(np.arange(128) < 64, -1.0, 1.0).astype(np.float32)
    sinM = np.ascontiguousarray(sinT * sign[:, None])

    # wide causal mask: maskW[p, u] = 1 iff p <= u - 384
    p_idx = np.arange(128)[:, None]
    u_idx = np.arange(896)[None, :]
    maskW = (p_idx <= u_idx - 384).astype(BF)

    ident = np.eye(128, dtype=np.float32)
    ones_c = np.ones((128, 1), np.float32).astype(BF)
    ones_r = np.ones((1, 128), np.float32)

    in_maps_a = []
    for c in range(N_CORES):
        g, bh = c // 2, c % 2
        in_maps_a.append({
            "xT": np.ascontiguousarray(xr[:, :, bh * TB:(bh + 1) * TB]).astype(BF),
            "wq": _to_pkto(Wq[:, 512 * g:512 * g + 512]).astype(BF),
            "wk": _to_pkto(Wk[:, 128 * g:128 * g + 128]).astype(BF),
            "wv": _to_pkto(Wv[:, 128 * g:128 * g + 128]).astype(BF),
            "cosT": cosT,
            "sinM": sinM,
            "maskW": maskW,
            "ident_in": ident,
            "ones_in": ones_c,
            "onesr_in": ones_r,
        })
    wo_r = _to_pkto(Wo).astype(BF)
    return in_maps_a, wo_r


def kernel(x, position_ids, Wq, Wk, Wv, Wo, _trace=False, _trace_kwargs=None):
    nca, ncb = _get_kernels()
    in_maps_a, wo_r = _prep_inputs(x, position_ids, Wq, Wk, Wv, Wo)

    kw = dict(trace=True, **(_trace_kwargs or {})) if _trace else {}
    res_a = run_bass_kernel_spmd(nca, in_maps_a, list(range(N_CORES)), **kw)

    # reassemble: Y[head, d, tok_global]
    Y = np.empty((16, 128, NTOK), dtype=BF)
    for c in range(N_CORES):
        g, bh = c // 2, c % 2
        yc = res_a.results[c]["y"]                    # [512, 4096] bf16
        for j in range(4):
            Y[4 * g + j, :, bh * TB:(bh + 1) * TB] = yc[128 * j:128 * (j + 1), :]
    in_maps_b = [{
        "ya": np.ascontiguousarray(
            Y[:, :, TOK_PER_CORE * c:TOK_PER_CORE * (c + 1)].transpose(1, 0, 2)),
        "wo": wo_r,
    } for c in range(N_CORES)]
    res_b = run_bass_kernel_spmd(ncb, in_maps_b, list(range(N_CORES)), **kw)
    out = np.concatenate([res_b.results[c]["outp"] for c in range(N_CORES)], axis=0)
    out = out.reshape(B, T, D).astype(np.float32)
    if _trace:
        return out, res_a, res_b
    return out


# revision 8
# speedup vs baseline: 1.0941x; 1.0941x over previous
"""Causal self-attention (GQA + RoPE) on 8 trn2 NeuronCores via Bass/Tile.

Sharding v2: core c = (g, bh) with g = c//2 the kv-group and bh = c%2 the
batch half. Each core projects Q for its 4 q-heads and K/V for its kv head,
over its 2 batches only — so K/V projections are computed exactly once
across the mesh (the old layout duplicated them 2x) and the x DMA halves.
o_proj stays token-parallel in a second kernel (host reslices y between the
two launches; all FLOPs on device).

Numerics: bf16 operands into the PE everywhere (same 1 cycle/row as f32r on
TRN2, but half the SBUF/DMA bytes and 2x DVE throughput), fp32 PSUM
accumulation. Softmax without max-subtraction. Softmax denominators: exp
tiles are tree-summed in groups of 4 on the DVE (bf16, 2x mode) and only one
ones-matmul per group hits the PE (vs one per tile before) — the PE was the
bottleneck engine, the DVE has slack.

Shapes hardcoded for B=4, T=2048, D=2048, 16 heads x 128, 4 kv heads x 128.
"""
import numpy as np
import ml_dtypes

import concourse.bacc as bacc
import concourse.mybir as mybir
from concourse.tile import TileContext
from concourse.bass_utils import run_bass_kernel_spmd

N_CORES = 8
B, T, D = 4, 2048, 2048
N_HEAD, N_KV, HD = 16, 4, 128
NTOK = B * T                      # 8192
CHUNK = 512
TOK_PER_CORE = NTOK // N_CORES    # 1024
TB = 2 * T                        # tokens per core in kernel A (2 batches)
SCALE = float(1.0 / np.sqrt(128.0))
ROPE_THETA = 10000.0

F32 = mybir.dt.float32
F32R = mybir.dt.float32r
BF16 = mybir.dt.bfloat16
BF = ml_dtypes.bfloat16


def build_kernel_a():
    nc = bacc.Bacc("TRN2", target_bir_lowering=False, debug=False,
                   num_devices=N_CORES, name="attn_a")
    xT = nc.dram_tensor("xT", [128, 16, TB], BF16, kind="ExternalInput")
    wq = nc.dram_tensor("wq", [128, 16, 512], BF16, kind="ExternalInput")
    wk = nc.dram_tensor("wk", [128, 16, 128], BF16, kind="ExternalInput")
    wv = nc.dram_tensor("wv", [128, 16, 128], BF16, kind="ExternalInput")
    cosT = nc.dram_tensor("cosT", [128, T], F32, kind="ExternalInput")
    sinM = nc.dram_tensor("sinM", [128, T], F32, kind="ExternalInput")
    maskW = nc.dram_tensor("maskW", [128, 896], BF16, kind="ExternalInput")
    ident_in = nc.dram_tensor("ident_in", [128, 128], F32, kind="ExternalInput")
    ones_in = nc.dram_tensor("ones_in", [128, 1], BF16, kind="ExternalInput")
    onesr_in = nc.dram_tensor("onesr_in", [1, 128], F32R, kind="ExternalInput")
    y = nc.dram_tensor("y", [512, TB], BF16, kind="ExternalOutput")

    with TileContext(nc) as tc:
        with tc.tile_pool(name="wpool", bufs=1) as wpool, \
             tc.tile_pool(name="xpool", bufs=8) as xpool, \
             tc.tile_pool(name="tpool", bufs=3) as tpool, \
             tc.tile_pool(name="qkv", bufs=2) as qkv, \
             tc.tile_pool(name="ep", bufs=8) as ep, \
             tc.tile_pool(name="gp", bufs=6) as gp, \
             tc.tile_pool(name="yu", bufs=4) as yu, \
             tc.tile_pool(name="su", bufs=4) as su, \
             tc.tile_pool(name="yp", bufs=3) as yp, \
             tc.tile_pool(name="psum", bufs=1, space="PSUM") as pp:
            # DMA issue order = service order: first proj (K of batch 0)
            # needs wk + the first x chunk, so those go first.
            wk_sb = wpool.tile([128, 16, 128], BF16)
            nc.sync.dma_start(out=wk_sb[:], in_=wk[:])
            xq0 = []
            for qtr in range(4):
                t = xpool.tile([128, 4, CHUNK], BF16, name="xq")
                nc.sync.dma_start(out=t[:], in_=xT[:, 4 * qtr:4 * qtr + 4, 0:CHUNK])
                xq0.append(t)
            wq_sb = wpool.tile([128, 16, 512], BF16)
            nc.sync.dma_start(out=wq_sb[:], in_=wq[:])
            wv_sb = wpool.tile([128, 16, 128], BF16)
            nc.sync.dma_start(out=wv_sb[:], in_=wv[:])
            cos_sb = wpool.tile([128, T], F32)
            nc.sync.dma_start(out=cos_sb[:], in_=cosT[:])
            sin_sb = wpool.tile([128, T], F32)
            nc.sync.dma_start(out=sin_sb[:], in_=sinM[:])
            id_sb = wpool.tile([128, 128], F32)
            nc.sync.dma_start(out=id_sb[:], in_=ident_in[:])
            mask_sb = wpool.tile([128, 896], BF16)
            nc.sync.dma_start(out=mask_sb[:], in_=maskW[:])
            ones_sb = wpool.tile([128, 1], BF16)
            nc.sync.dma_start(out=ones_sb[:], in_=ones_in[:])
            onesr_sb = wpool.tile([1, 128], F32R)
            nc.sync.dma_start(out=onesr_sb[:], in_=onesr_in[:])

            pending = []

            def flush_one():
                y_u, rrow, h, b, qc = pending.pop(0)
                col0 = b * T + qc * CHUNK
                b_ps = pp.tile([128, CHUNK], F32, name="b_ps", bufs=1)
                nc.tensor.matmul(b_ps[:], onesr_sb[:], rrow[:], start=True, stop=True)
                y_sb = yp.tile([128, CHUNK], BF16, name="y_sb")
                nc.vector.tensor_mul(y_sb[:], y_u[:], b_ps[:])
                nc.sync.dma_start(out=y[h * 128:(h + 1) * 128, col0:col0 + CHUNK],
                                  in_=y_sb[:])

            for b in range(2):
                # ---- projections + rope for local batch b ----
                qb = [qkv.tile([128, T], BF16, name=f"qb{h}") for h in range(4)]
                kb = qkv.tile([128, T], BF16, name="kb")
                vtb = qkv.tile([128, 16, 128], BF16, name="vtb")
                for cc in range(4):
                    c0 = b * T + cc * CHUNK
                    tcol = cc * CHUNK
                    if b == 0 and cc == 0:
                        xq = xq0
                    else:
                        xq = []
                        for qtr in range(4):
                            t = xpool.tile([128, 4, CHUNK], BF16, name="xq")
                            nc.sync.dma_start(
                                out=t[:], in_=xT[:, 4 * qtr:4 * qtr + 4, c0:c0 + CHUNK])
                            xq.append(t)

                    def proj(w_sb, off):
                        ps2 = pp.tile([128, 2 * CHUNK], F32, name="s2", bufs=2)
                        ps = ps2[:, 0:CHUNK]
                        for kt in range(16):
                            nc.tensor.matmul(ps, w_sb[:, kt, off:off + 128],
                                             xq[kt // 4][:, kt % 4, :],
                                             start=(kt == 0), stop=(kt == 15))
                        return ps

                    def rope(ps, dst):
                        # DVE reads the proj psum directly (fp32, 1x) exactly
                        # like the proven baseline; only dst is bf16.
                        t1 = tpool.tile([128, CHUNK], F32, name="t1")
                        t2 = tpool.tile([128, CHUNK], F32, name="t2")
                        nc.vector.tensor_mul(t1[:], ps[:], cos_sb[:, tcol:tcol + CHUNK])
                        nc.vector.tensor_mul(t2[0:64, :], ps[64:128, :],
                                             sin_sb[0:64, tcol:tcol + CHUNK])
                        nc.vector.tensor_mul(t2[64:128, :], ps[0:64, :],
                                             sin_sb[64:128, tcol:tcol + CHUNK])
                        nc.vector.tensor_add(dst, t1[:], t2[:])

                    rope(proj(wk_sb, 0), kb[:, tcol:tcol + CHUNK])

                    ps_v = proj(wv_sb, 0)
                    vtmp = tpool.tile([128, CHUNK], F32, name="vtmp")
                    nc.scalar.copy(vtmp[:], ps_v)
                    pt = pp.tile([128, CHUNK], F32, name="b_ps", bufs=1)
                    for j in range(4):
                        nc.tensor.transpose(pt[:, j * 128:(j + 1) * 128],
                                            vtmp[:, j * 128:(j + 1) * 128], id_sb[:])
                    for j in range(4):
                        nc.scalar.copy(vtb[:, 4 * cc + j, :], pt[:, j * 128:(j + 1) * 128])

                    for h in range(4):
                        rope(proj(wq_sb, 128 * h), qb[h][:, tcol:tcol + CHUNK])

                # ---- attention for local batch b ----
                for h in range(4):
                    for qc in range(4):
                        while len(pending) > 2:
                            flush_one()
                        y_ps = pp.tile([128, CHUNK], F32, name="y_ps", bufs=2)
                        sum_ps = pp.tile([1, CHUNK], F32, name="sum_ps", bufs=1)
                        for grp in range(qc + 1):
                            e2s = []
                            for p in range(2):
                                s2 = pp.tile([128, 2 * CHUNK], F32, name="s2", bufs=2)
                                e2 = ep.tile([128, 2 * CHUNK], BF16, name="e_sb")
                                for jj in range(2):
                                    kt = 4 * grp + 2 * p + jj
                                    nc.tensor.matmul(
                                        s2[:, jj * CHUNK:(jj + 1) * CHUNK],
                                        kb[:, kt * 128:(kt + 1) * 128],
                                        qb[h][:, qc * CHUNK:(qc + 1) * CHUNK],
                                        start=True, stop=True)
                                nc.scalar.activation(e2[:], s2[:],
                                                     mybir.ActivationFunctionType.Exp,
                                                     bias=0.0, scale=SCALE)
                                for jj in range(2):
                                    kt = 4 * grp + 2 * p + jj
                                    j = 2 * p + jj
                                    eh = e2[:, jj * CHUNK:(jj + 1) * CHUNK]
                                    if grp == qc:
                                        off = 384 - 128 * j
                                        nc.vector.tensor_mul(eh, eh,
                                                             mask_sb[:, off:off + CHUNK])
                                    nc.tensor.matmul(y_ps[:], vtb[:, kt, :], eh,
                                                     start=(kt == 0),
                                                     stop=(kt == 4 * qc + 3))
                                e2s.append(e2)
                            # group denominator: halves-adds on the idle Pool
                            # engine (off the exp->PV critical path), final
                            # combine on DVE, one ones-matmul per group.
                            g01 = gp.tile([128, CHUNK], BF16, name="g")
                            nc.vector.tensor_add(g01[:], e2s[0][:, 0:CHUNK],
                                                 e2s[0][:, CHUNK:2 * CHUNK])
                            g23 = gp.tile([128, CHUNK], BF16, name="g")
                            nc.vector.tensor_add(g23[:], e2s[1][:, 0:CHUNK],
                                                 e2s[1][:, CHUNK:2 * CHUNK])
                            gs = gp.tile([128, CHUNK], BF16, name="g")
                            nc.vector.tensor_add(gs[:], g01[:], g23[:])
                            nc.tensor.matmul(sum_ps[:], ones_sb[:], gs[:],
                                             start=(grp == 0), stop=(grp == qc))
                        y_u = yu.tile([128, CHUNK], BF16, name="y_u")
                        nc.scalar.copy(y_u[:], y_ps[:])
                        lrow = su.tile([1, CHUNK], F32, name="lrow", bufs=2)
                        nc.vector.reciprocal_approx_fast(out=lrow[:], in_=sum_ps[:])
                        rrow = su.tile([1, CHUNK], F32R, name="rrow")
                        nc.scalar.copy(rrow[:], lrow[:])
                        pending.append((y_u, rrow, h, b, qc))
            while pending:
                flush_one()
    nc.compile()
    return nc


def build_kernel_b():
    nc = bacc.Bacc("TRN2", target_bir_lowering=False, debug=False,
                   num_devices=N_CORES, name="attn_b")
    ya = nc.dram_tensor("ya", [128, 16, TOK_PER_CORE], BF16, kind="ExternalInput")
    wo = nc.dram_tensor("wo", [128, 16, D], BF16, kind="ExternalInput")
    outp = nc.dram_tensor("outp", [TOK_PER_CORE, D], F32, kind="ExternalOutput")
    NTT = TOK_PER_CORE // 128          # 8
    with TileContext(nc) as tc:
        with tc.tile_pool(name="yap", bufs=1) as yap, \
             tc.tile_pool(name="wop", bufs=18) as wop, \
             tc.tile_pool(name="obp", bufs=3) as obp, \
             tc.tile_pool(name="pb", bufs=4, space="PSUM") as pb:
            # fine-grained DMA: first matmul only needs ya tile 0 + one
            # [128,512] weight slice (~0.6 MB) instead of 1.5 MB -> PE
            # starts ~10us earlier.
            ya_t = []
            t = yap.tile([128, 16, 128], BF16, name="yat0")
            nc.sync.dma_start(out=t[:], in_=ya[:, :, 0:128])
            ya_t.append(t)
            w0 = []
            for kt in range(16):
                w = wop.tile([128, 512], BF16, name="wt")
                nc.sync.dma_start(out=w[:], in_=wo[:, kt, 0:512])
                w0.append(w)
                if kt < NTT - 1:
                    t = yap.tile([128, 16, 128], BF16, name=f"yat{kt + 1}")
                    nc.sync.dma_start(out=t[:],
                                      in_=ya[:, :, (kt + 1) * 128:(kt + 2) * 128])
                    ya_t.append(t)
            for oc in range(4):
                if oc == 0:
                    wt = w0
                else:
                    wt = []
                    for kt in range(16):
                        w = wop.tile([128, 512], BF16, name="wt")
                        nc.sync.dma_start(out=w[:],
                                          in_=wo[:, kt, oc * 512:(oc + 1) * 512])
                        wt.append(w)
                for tt in range(NTT):
                    ps = pb.tile([128, 512], F32, name="ps")
                    for kt in range(16):
                        nc.tensor.matmul(ps[:], ya_t[tt][:, kt, :], wt[kt][:],
                                         start=(kt == 0), stop=(kt == 15))
                    ob = obp.tile([128, 512], F32, name="ob")
                    nc.scalar.copy(ob[:], ps[:])
                    nc.sync.dma_start(
                        out=outp[tt * 128:(tt + 1) * 128, oc * 512:(oc + 1) * 512],
                        in_=ob[:])
    nc.compile()
    return nc


_cache = {}


def _get_kernels():
    if "a" not in _cache:
        _cache["a"] = build_kernel_a()
        _cache["b"] = build_kernel_b()
    return _cache["a"], _cache["b"]


def _to_pkto(w):
    # (D, O) -> (128, D//128, O): partition-major layout matching SBUF tiles
    Dd, O = w.shape
    return np.ascontiguousarray(w.reshape(Dd // 128, 128, O).transpose(1, 0, 2))


def _prep_inputs(x, position_ids, Wq, Wk, Wv, Wo):
    x = np.ascontiguousarray(np.asarray(x), dtype=np.float32)
    pos = np.asarray(position_ids).astype(np.float32)
    Wq = np.asarray(Wq, dtype=np.float32)
    Wk = np.asarray(Wk, dtype=np.float32)
    Wv = np.asarray(Wv, dtype=np.float32)
    Wo = np.asarray(Wo, dtype=np.float32)

    xr = x.reshape(NTOK, D).T.reshape(16, 128, NTOK).transpose(1, 0, 2)  # [128,16,NTOK]

    inv = (1.0 / (ROPE_THETA ** (np.arange(0, HD, 2, dtype=np.float32) / HD))).astype(np.float32)
    freqs = np.outer(pos, inv).astype(np.float32)          # (T, 64)
    emb = np.concatenate([freqs, freqs], axis=1)           # (T, 128)
    cosT = np.ascontiguousarray(np.cos(emb).T)             # (128, T)
    sinT = np.sin(emb).T
    sign = np.where(np.arange(128) < 64, -1.0, 1.0).astype(np.float32)
    sinM = np.ascontiguousarray(sinT * sign[:, None])

    # wide causal mask: maskW[p, u] = 1 iff p <= u - 384
    p_idx = np.arange(128)[:, None]
    u_idx = np.arange(896)[None, :]
    maskW = (p_idx <= u_idx - 384).astype(BF)

    ident = np.eye(128, dtype=np.float32)
    ones_c = np.ones((128, 1), np.float32).astype(BF)
    ones_r = np.ones((1, 128), np.float32)

    in_maps_a = []
    for c in range(N_CORES):
        g, bh = c // 2, c % 2
        in_maps_a.append({
            "xT": np.ascontiguousarray(xr[:, :, bh * TB:(bh + 1) * TB]).astype(BF),
            "wq": _to_pkto(Wq[:, 512 * g:512 * g + 512]).astype(BF),
            "wk": _to_pkto(Wk[:, 128 * g:128 * g + 128]).astype(BF),
            "wv": _to_pkto(Wv[:, 128 * g:128 * g + 128]).astype(BF),
            "cosT": cosT,
            "sinM": sinM,
            "maskW": maskW,
            "ident_in": ident,
            "ones_in": ones_c,
            "onesr_in": ones_r,
        })
    wo_r = _to_pkto(Wo).astype(BF)
    return in_maps_a, wo_r


def kernel(x, position_ids, Wq, Wk, Wv, Wo, _trace=False, _trace_kwargs=None):
    nca, ncb = _get_kernels()
    in_maps_a, wo_r = _prep_inputs(x, position_ids, Wq, Wk, Wv, Wo)

    kw = dict(trace=True, **(_trace_kwargs or {})) if _trace else {}
    res_a = run_bass_kernel_spmd(nca, in_maps_a, list(range(N_CORES)), **kw)

    # reassemble: Y[head, d, tok_global]
    Y = np.empty((16, 128, NTOK), dtype=BF)
    for c in range(N_CORES):
        g, bh = c // 2, c % 2
        yc = res_a.results[c]["y"]                    # [512, 4096] bf16
        for j in range(4):
            Y[4 * g + j, :, bh * TB:(bh + 1) * TB] = yc[128 * j:128 * (j + 1), :]
    in_maps_b = [{
        "ya": np.ascontiguousarray(
            Y[:, :, TOK_PER_CORE * c:TOK_PER_CORE * (c + 1)].transpose(1, 0, 2)),
        "wo": wo_r,
    } for c in range(N_CORES)]
    res_b = run_bass_kernel_spmd(ncb, in_maps_b, list(range(N_CORES)), **kw)
    out = np.concatenate([res_b.results[c]["outp"] for c in range(N_CORES)], axis=0)
    out = out.reshape(B, T, D).astype(np.float32)
    if _trace:
        return out, res_a, res_b
    return out


# revision 9
# speedup vs baseline: 1.1381x; 1.0402x over previous
"""Causal self-attention (GQA + RoPE) on 8 trn2 NeuronCores via Bass/Tile.

Sharding v2: core c = (g, bh) with g = c//2 the kv-group and bh = c%2 the
batch half. Each core projects Q for its 4 q-heads and K/V for its kv head,
over its 2 batches only — so K/V projections are computed exactly once
across the mesh (the old layout duplicated them 2x) and the x DMA halves.
o_proj stays token-parallel in a second kernel (host reslices y between the
two launches; all FLOPs on device).

Numerics: bf16 operands into the PE everywhere (same 1 cycle/row as f32r on
TRN2, but half the SBUF/DMA bytes and 2x DVE throughput), fp32 PSUM
accumulation. Softmax without max-subtraction. Softmax denominators: exp
tiles are tree-summed in groups of 4 on the DVE (bf16, 2x mode) and only one
ones-matmul per group hits the PE (vs one per tile before) — the PE was the
bottleneck engine, the DVE has slack.

Shapes hardcoded for B=4, T=2048, D=2048, 16 heads x 128, 4 kv heads x 128.
"""
import numpy as np
import ml_dtypes

import concourse.bacc as bacc
import concourse.mybir as mybir
from concourse.tile import TileContext
from concourse.bass_utils import run_bass_kernel_spmd

N_CORES = 8
B, T, D = 4, 2048, 2048
N_HEAD, N_KV, HD = 16, 4, 128
NTOK = B * T                      # 8192
CHUNK = 512
TOK_PER_CORE = NTOK // N_CORES    # 1024
TB = 2 * T                        # tokens per core in kernel A (2 batches)
SCALE = float(1.0 / np.sqrt(128.0))
ROPE_THETA = 10000.0

F32 = mybir.dt.float32
F32R = mybir.dt.float32r
BF16 = mybir.dt.bfloat16
BF = ml_dtypes.bfloat16


def build_kernel_a():
    nc = bacc.Bacc("TRN2", target_bir_lowering=False, debug=False,
                   num_devices=N_CORES, name="attn_a")
    xT = nc.dram_tensor("xT", [128, 16, TB], BF16, kind="ExternalInput")
    wq = nc.dram_tensor("wq", [128, 16, 512], BF16, kind="ExternalInput")
    wk = nc.dram_tensor("wk", [128, 16, 128], BF16, kind="ExternalInput")
    wv = nc.dram_tensor("wv", [128, 16, 128], BF16, kind="ExternalInput")
    cosT = nc.dram_tensor("cosT", [128, T], F32, kind="ExternalInput")
    sinM = nc.dram_tensor("sinM", [128, T], F32, kind="ExternalInput")
    maskW = nc.dram_tensor("maskW", [128, 896], BF16, kind="ExternalInput")
    ident_in = nc.dram_tensor("ident_in", [128, 128], F32, kind="ExternalInput")
    ones_in = nc.dram_tensor("ones_in", [128, 1], BF16, kind="ExternalInput")
    onesr_in = nc.dram_tensor("onesr_in", [1, 128], F32R, kind="ExternalInput")
    y = nc.dram_tensor("y", [512, TB], BF16, kind="ExternalOutput")

    with TileContext(nc) as tc:
        with tc.tile_pool(name="wpool", bufs=1) as wpool, \
             tc.tile_pool(name="xpool", bufs=8) as xpool, \
             tc.tile_pool(name="tpool", bufs=3) as tpool, \
             tc.tile_pool(name="qkv", bufs=2) as qkv, \
             tc.tile_pool(name="ep", bufs=8) as ep, \
             tc.tile_pool(name="gp", bufs=6) as gp, \
             tc.tile_pool(name="yu", bufs=4) as yu, \
             tc.tile_pool(name="su", bufs=4) as su, \
             tc.tile_pool(name="yp", bufs=3) as yp, \
             tc.tile_pool(name="psum", bufs=1, space="PSUM") as pp:
            # DMA issue order = service order: first proj (K of batch 0)
            # needs wk + the first x chunk, so those go first.
            wk_sb = wpool.tile([128, 16, 128], BF16)
            nc.sync.dma_start(out=wk_sb[:], in_=wk[:])
            xq0 = []
            for qtr in range(4):
                t = xpool.tile([128, 4, CHUNK], BF16, name="xq")
                nc.sync.dma_start(out=t[:], in_=xT[:, 4 * qtr:4 * qtr + 4, 0:CHUNK])
                xq0.append(t)
            wq_sb = wpool.tile([128, 16, 512], BF16)
            nc.sync.dma_start(out=wq_sb[:], in_=wq[:])
            wv_sb = wpool.tile([128, 16, 128], BF16)
            nc.sync.dma_start(out=wv_sb[:], in_=wv[:])
            cos_sb = wpool.tile([128, T], F32)
            nc.sync.dma_start(out=cos_sb[:], in_=cosT[:])
            sin_sb = wpool.tile([128, T], F32)
            nc.sync.dma_start(out=sin_sb[:], in_=sinM[:])
            id_sb = wpool.tile([128, 128], F32)
            nc.sync.dma_start(out=id_sb[:], in_=ident_in[:])
            mask_sb = wpool.tile([128, 896], BF16)
            nc.sync.dma_start(out=mask_sb[:], in_=maskW[:])
            ones_sb = wpool.tile([128, 1], BF16)
            nc.sync.dma_start(out=ones_sb[:], in_=ones_in[:])
            onesr_sb = wpool.tile([1, 128], F32R)
            nc.sync.dma_start(out=onesr_sb[:], in_=onesr_in[:])

            pending = []

            def flush_one():
                y_u, rrow, h, b, qc = pending.pop(0)
                col0 = b * T + qc * CHUNK
                b_ps = pp.tile([128, CHUNK], F32, name="b_ps", bufs=1)
                nc.tensor.matmul(b_ps[:], onesr_sb[:], rrow[:], start=True, stop=True)
                y_sb = yp.tile([128, CHUNK], BF16, name="y_sb")
                nc.vector.tensor_mul(y_sb[:], y_u[:], b_ps[:])
                nc.sync.dma_start(out=y[h * 128:(h + 1) * 128, col0:col0 + CHUNK],
                                  in_=y_sb[:])

            for b in range(2):
                # ---- projections + rope for local batch b ----
                qb = [qkv.tile([128, T], BF16, name=f"qb{h}") for h in range(4)]
                kb = qkv.tile([128, T], BF16, name="kb")
                vtb = qkv.tile([128, 16, 128], BF16, name="vtb")
                for cc in range(4):
                    c0 = b * T + cc * CHUNK
                    tcol = cc * CHUNK
                    if b == 0 and cc == 0:
                        xq = xq0
                    else:
                        xq = []
                        for qtr in range(4):
                            t = xpool.tile([128, 4, CHUNK], BF16, name="xq")
                            nc.sync.dma_start(
                                out=t[:], in_=xT[:, 4 * qtr:4 * qtr + 4, c0:c0 + CHUNK])
                            xq.append(t)

                    def proj(w_sb, off):
                        ps2 = pp.tile([128, 2 * CHUNK], F32, name="s2", bufs=2)
                        ps = ps2[:, 0:CHUNK]
                        for kt in range(16):
                            nc.tensor.matmul(ps, w_sb[:, kt, off:off + 128],
                                             xq[kt // 4][:, kt % 4, :],
                                             start=(kt == 0), stop=(kt == 15))
                        return ps

                    def rope(ps, dst):
                        # DVE reads the proj psum directly (fp32, 1x) exactly
                        # like the proven baseline; only dst is bf16.
                        t1 = tpool.tile([128, CHUNK], F32, name="t1")
                        t2 = tpool.tile([128, CHUNK], F32, name="t2")
                        nc.vector.tensor_mul(t1[:], ps[:], cos_sb[:, tcol:tcol + CHUNK])
                        nc.vector.tensor_mul(t2[0:64, :], ps[64:128, :],
                                             sin_sb[0:64, tcol:tcol + CHUNK])
                        nc.vector.tensor_mul(t2[64:128, :], ps[0:64, :],
                                             sin_sb[64:128, tcol:tcol + CHUNK])
                        nc.vector.tensor_add(dst, t1[:], t2[:])

                    rope(proj(wk_sb, 0), kb[:, tcol:tcol + CHUNK])

                    ps_v = proj(wv_sb, 0)
                    vtmp = tpool.tile([128, CHUNK], F32, name="vtmp")
                    nc.scalar.copy(vtmp[:], ps_v)
                    pt = pp.tile([128, CHUNK], F32, name="b_ps", bufs=1)
                    for j in range(4):
                        nc.tensor.transpose(pt[:, j * 128:(j + 1) * 128],
                                            vtmp[:, j * 128:(j + 1) * 128], id_sb[:])
                    for j in range(4):
                        nc.scalar.copy(vtb[:, 4 * cc + j, :], pt[:, j * 128:(j + 1) * 128])

                    for h in range(4):
                        rope(proj(wq_sb, 128 * h), qb[h][:, tcol:tcol + CHUNK])

                # ---- attention for local batch b ----
                for h in range(4):
                    for qc in range(4):
                        while len(pending) > 2:
                            flush_one()
                        y_ps = pp.tile([128, CHUNK], F32, name="y_ps", bufs=2)
                        sum_ps = pp.tile([1, CHUNK], F32, name="sum_ps", bufs=1)
                        for grp in range(qc + 1):
                            e2s = []
                            for p in range(2):
                                s2 = pp.tile([128, 2 * CHUNK], F32, name="s2", bufs=2)
                                e2 = ep.tile([128, 2 * CHUNK], BF16, name="e_sb")
                                for jj in range(2):
                                    kt = 4 * grp + 2 * p + jj
                                    nc.tensor.matmul(
                                        s2[:, jj * CHUNK:(jj + 1) * CHUNK],
                                        kb[:, kt * 128:(kt + 1) * 128],
                                        qb[h][:, qc * CHUNK:(qc + 1) * CHUNK],
                                        start=True, stop=True)
                                nc.scalar.activation(e2[:], s2[:],
                                                     mybir.ActivationFunctionType.Exp,
                                                     bias=0.0, scale=SCALE)
                                for jj in range(2):
                                    kt = 4 * grp + 2 * p + jj
                                    j = 2 * p + jj
                                    eh = e2[:, jj * CHUNK:(jj + 1) * CHUNK]
                                    if grp == qc:
                                        off = 384 - 128 * j
                                        nc.vector.tensor_mul(eh, eh,
                                                             mask_sb[:, off:off + CHUNK])
                                    nc.tensor.matmul(y_ps[:], vtb[:, kt, :], eh,
                                                     start=(kt == 0),
                                                     stop=(kt == 4 * qc + 3))
                                e2s.append(e2)
                            # group denominator: halves-adds on the idle Pool
                            # engine (off the exp->PV critical path), final
                            # combine on DVE, one ones-matmul per group.
                            g01 = gp.tile([128, CHUNK], BF16, name="g")
                            nc.vector.tensor_add(g01[:], e2s[0][:, 0:CHUNK],
                                                 e2s[0][:, CHUNK:2 * CHUNK])
                            g23 = gp.tile([128, CHUNK], BF16, name="g")
                            nc.vector.tensor_add(g23[:], e2s[1][:, 0:CHUNK],
                                                 e2s[1][:, CHUNK:2 * CHUNK])
                            gs = gp.tile([128, CHUNK], BF16, name="g")
                            nc.vector.tensor_add(gs[:], g01[:], g23[:])
                            nc.tensor.matmul(sum_ps[:], ones_sb[:], gs[:],
                                             start=(grp == 0), stop=(grp == qc))
                        y_u = yu.tile([128, CHUNK], BF16, name="y_u")
                        nc.scalar.copy(y_u[:], y_ps[:])
                        lrow = su.tile([1, CHUNK], F32, name="lrow", bufs=2)
                        nc.vector.reciprocal_approx_fast(out=lrow[:], in_=sum_ps[:])
                        rrow = su.tile([1, CHUNK], F32R, name="rrow")
                        nc.scalar.copy(rrow[:], lrow[:])
                        pending.append((y_u, rrow, h, b, qc))
            while pending:
                flush_one()
    nc.compile()
    return nc


def build_kernel_b():
    nc = bacc.Bacc("TRN2", target_bir_lowering=False, debug=False,
                   num_devices=N_CORES, name="attn_b")
    ya = nc.dram_tensor("ya", [128, 16, TOK_PER_CORE], BF16, kind="ExternalInput")
    wo = nc.dram_tensor("wo", [128, 16, D], BF16, kind="ExternalInput")
    outp = nc.dram_tensor("outp", [TOK_PER_CORE, D], F32, kind="ExternalOutput")
    NTT = TOK_PER_CORE // 128          # 8
    with TileContext(nc) as tc:
        with tc.tile_pool(name="yap", bufs=1) as yap, \
             tc.tile_pool(name="wop", bufs=2) as wop, \
             tc.tile_pool(name="obp", bufs=3) as obp, \
             tc.tile_pool(name="pb", bufs=4, space="PSUM") as pb:
            wlo0 = wop.tile([128, 8, 512], BF16, name="wlo")
            nc.sync.dma_start(out=wlo0[:], in_=wo[:, 0:8, 0:512])
            ya_t = []
            for tt in range(NTT):
                t = yap.tile([128, 16, 128], BF16, name=f"yat{tt}")
                nc.sync.dma_start(out=t[:], in_=ya[:, :, tt * 128:(tt + 1) * 128])
                ya_t.append(t)
                if tt == 0:
                    whi0 = wop.tile([128, 8, 512], BF16, name="whi")
                    nc.sync.dma_start(out=whi0[:], in_=wo[:, 8:16, 0:512])
            for oc in range(4):
                if oc == 0:
                    wlo, whi = wlo0, whi0
                else:
                    wlo = wop.tile([128, 8, 512], BF16, name="wlo")
                    nc.sync.dma_start(out=wlo[:], in_=wo[:, 0:8, oc * 512:(oc + 1) * 512])
                    whi = wop.tile([128, 8, 512], BF16, name="whi")
                    nc.sync.dma_start(out=whi[:], in_=wo[:, 8:16, oc * 512:(oc + 1) * 512])
                for tt in range(NTT):
                    ps = pb.tile([128, 512], F32, name="ps")
                    for kt in range(16):
                        w = wlo if kt < 8 else whi
                        nc.tensor.matmul(ps[:], ya_t[tt][:, kt, :], w[:, kt % 8, :],
                                         start=(kt == 0), stop=(kt == 15))
                    ob = obp.tile([128, 512], F32, name="ob")
                    nc.scalar.copy(ob[:], ps[:])
                    nc.sync.dma_start(
                        out=outp[tt * 128:(tt + 1) * 128, oc * 512:(oc + 1) * 512],
                        in_=ob[:])
    nc.compile()
    return nc


_cache = {}


def _get_kernels():
    if "a" not in _cache:
        _cache["a"] = build_kernel_a()
        _cache["b"] = build_kernel_b()
    return _cache["a"], _cache["b"]


def _to_pkto(w):
    # (D, O) -> (128, D//128, O): partition-major layout matching SBUF tiles
    Dd, O = w.shape
    return np.ascontiguousarray(w.reshape(Dd // 128, 128, O).transpose(1, 0, 2))


def _prep_inputs(x, position_ids, Wq, Wk, Wv, Wo):
    x = np.ascontiguousarray(np.asarray(x), dtype=np.float32)
    pos = np.asarray(position_ids).astype(np.float32)
    Wq = np.asarray(Wq, dtype=np.float32)
    Wk = np.asarray(Wk, dtype=np.float32)
    Wv = np.asarray(Wv, dtype=np.float32)
    Wo = np.asarray(Wo, dtype=np.float32)

    xr = x.reshape(NTOK, D).T.reshape(16, 128, NTOK).transpose(1, 0, 2)  # [128,16,NTOK]

    inv = (1.0 / (ROPE_THETA ** (np.arange(0, HD, 2, dtype=np.float32) / HD))).astype(np.float32)
    freqs = np.outer(pos, inv).astype(np.float32)          # (T, 64)
    emb = np.concatenate([freqs, freqs], axis=1)           # (T, 128)
    cosT = np.ascontiguousarray(np.cos(emb).T)             # (128, T)
    sinT = np.sin(emb).T
    sign = np.where(np.arange(128) < 64, -1.0, 1.0).astype(np.float32)
    sinM = np.ascontiguousarray(sinT * sign[:, None])

    # wide causal mask: maskW[p, u] = 1 iff p <= u - 384
    p_idx = np.arange(128)[:, None]
    u_idx = np.arange(896)[None, :]
    maskW = (p_idx <= u_idx - 384).astype(BF)

    ident = np.eye(128, dtype=np.float32)
    ones_c = np.ones((128, 1), np.float32).astype(BF)
    ones_r = np.ones((1, 128), np.float32)

    in_maps_a = []
    for c in range(N_CORES):
        g, bh = c // 2, c % 2
        in_maps_a.append({
            "xT": np.ascontiguousarray(xr[:, :, bh * TB:(bh + 1) * TB]).astype(BF),
            "wq": _to_pkto(Wq[:, 512 * g:512 * g + 512]).astype(BF),
            "wk": _to_pkto(Wk[:, 128 * g:128 * g + 128]).astype(BF),
            "wv": _to_pkto(Wv[:, 128 * g:128 * g + 128]).astype(BF),
            "cosT": cosT,
            "sinM": sinM,
            "maskW": maskW,
            "ident_in": ident,
            "ones_in": ones_c,
            "onesr_in": ones_r,
        })
    wo_r = _to_pkto(Wo).astype(BF)
    return in_maps_a, wo_r


def kernel(x, position_ids, Wq, Wk, Wv, Wo, _trace=False, _trace_kwargs=None):
    nca, ncb = _get_kernels()
    in_maps_a, wo_r = _prep_inputs(x, position_ids, Wq, Wk, Wv, Wo)

    kw = dict(trace=True, **(_trace_kwargs or {})) if _trace else {}
    res_a = run_bass_kernel_spmd(nca, in_maps_a, list(range(N_CORES)), **kw)

    # reassemble: Y[head, d, tok_global]
    Y = np.empty((16, 128, NTOK), dtype=BF)
    for c in range(N_CORES):
        g, bh = c // 2, c % 2
        yc = res_a.results[c]["y"]                    # [512, 4096] bf16
        for j in range(4):
            Y[4 * g + j, :, bh * TB:(bh + 1) * TB] = yc[128 * j:128 * (j + 1), :]
    in_maps_b = [{
        "ya": np.ascontiguousarray(
            Y[:, :, TOK_PER_CORE * c:TOK_PER_CORE * (c + 1)].transpose(1, 0, 2)),
        "wo": wo_r,
    } for c in range(N_CORES)]
    res_b = run_bass_kernel_spmd(ncb, in_maps_b, list(range(N_CORES)), **kw)
    out = np.concatenate([res_b.results[c]["outp"] for c in range(N_CORES)], axis=0)
    out = out.reshape(B, T, D).astype(np.float32)
    if _trace:
        return out, res_a, res_b
    return out
